# revision 1
# baseline (speedup 1.0000x reference)
"""Trainium2 Bass kernel for the ragged_sequence segment-logits model.

Model (per node n, H=128):
    h   = silu(silu((xs[n]*W1@ws + xn[n]*W1@wn) + b1) @ W2.T + b2)   [N,H]
    node_scores = h @ w_score                                        [N]
per segment b (B=2048 segments of L=512 contiguous nodes):
    stop_node = mean(h[start : start+mean_len])                      [H]
    node_logits = log_softmax(node_scores[start : start+cand_len])
    stop_logits = log_softmax(stop_node @ W_stop.T)                  [2]
    out[b] = [node_logits + stop_logits[0], stop_logits[1]]          [L+1]

Sharding: data-parallel over segments, 256 segments per core on 8 cores.
Layout on device: H on partitions, nodes on the free axis ("hT" layout).
The embedding + first linear layer are fused host-side:
    h1pre = (W1 @ w_seed) outer xs + (W1 @ w_node) outer xn
so layer 1 is a single K=2 matmul from the raw scalar inputs.
node_scores are accumulated straight into segment-major PSUM layout
[seg, 512] using 128 sliding-window one-hot stationary matrices
(column b = w_score), so no repartitioning pass is needed.
Layer-2/score matmuls run in float32r (tf32, full PE rate); layer-1
inputs are bf16 so the [2, N] X transfer fits the ~2GB/s per-partition
SBUF DMA write bandwidth (fp32 made the X DMA chain the critical path).
Pipeline: 3-segment blocks, layer-2 lagged one block and scores two, so
ScalarE (the silu bottleneck, ~94% busy) never waits on TensorE.
Per-group epilogues keep their DVE/PE head inline but defer all Exp/Ln
to one batch at the end (a single ACT table-set switch); prologue
warmers preload the Silu table and un-throttle the PE clock gate.
"""

import sys
import numpy as np

for _p in ("/opt/trn_rl_repo", "/root/.axon_site/_ro/trn_rl_repo"):
    if _p not in sys.path:
        sys.path.insert(0, _p)

H = 128
B = 2048
L = 512
N = B * L
NCORES = 8
BC = B // NCORES          # segments per core
NC_NODES = BC * L         # nodes per core
SEGS_PER_BLOCK = 3        # segments per pipeline block (PSUM: 3+3+2 banks)
GROUP = 128               # segments per output group
NGROUP = BC // GROUP      # 2

_nc_cache = {}


def _silu_np(x):
    return x / (1.0 + np.exp(-x))


def _numpy_ref(x_seeds, x_nodes, w_seed, w_node, W1, b1, W2, b2, w_score,
               W_stop, indptr):
    """Exact fallback for irregular indptr (not expected to be hit)."""
    x_seeds = x_seeds.astype(np.float32)
    x_nodes = x_nodes.astype(np.float32)
    h = x_seeds[:, None] * w_seed[None, :] + x_nodes[:, None] * w_node[None, :]
    h = _silu_np(h @ W1.T + b1)
    h = _silu_np(h @ W2.T + b2)
    node_scores = h @ w_score
    starts = indptr[:, 0].astype(np.int64)
    mean_len = (indptr[:, 1] - indptr[:, 0]).astype(np.int64)
    cand_len = (indptr[:, 2] - indptr[:, 0]).astype(np.int64)
    pos = np.arange(L)
    seg = starts[:, None] + pos[None, :]
    h_seg = h[seg]
    mmask = pos[None, :] < mean_len[:, None]
    stop_node = (h_seg * mmask[..., None]).sum(axis=1) / mean_len[:, None]
    cmask = pos[None, :] < cand_len[:, None]
    scores = np.where(cmask, node_scores[seg], -np.inf)
    smax = scores.max(axis=1, keepdims=True)
    node_logits = scores - smax - np.log(
        np.exp(scores - smax).sum(axis=1, keepdims=True))
    sv = stop_node @ W_stop.T
    svmax = sv.max(axis=1, keepdims=True)
    stop_logits = sv - svmax - np.log(
        np.exp(sv - svmax).sum(axis=1, keepdims=True))
    return np.concatenate(
        [node_logits + stop_logits[:, 0:1], stop_logits[:, 1:2]],
        axis=1).astype(np.float32)


def _build(mean_w, nreps=1, probe=None):
    """Build the Bass program. mean_w: uniform mean-pool width (python int).
    nreps>1 repeats the whole compute pipeline (benchmarking only).
    probe: bench-only ablations/additions (None | 'small_act' | 'no_scores'
    | 'act2' | 'pe2' | 'dve2' | 'dma2')."""
    import concourse.tile as tile
    from concourse import bacc, mybir

    f32 = mybir.dt.float32
    f32r = mybir.dt.float32r
    AF = mybir.ActivationFunctionType
    AL = mybir.AluOpType

    nc = bacc.Bacc(None, target_bir_lowering=False, debug=False)

    bf16 = mybir.dt.bfloat16
    x_d = nc.dram_tensor("x2", [2, NC_NODES], bf16, kind="ExternalInput")
    a_d = nc.dram_tensor("a2", [2, H], bf16, kind="ExternalInput")
    w2t_d = nc.dram_tensor("w2t", [H, H], f32r, kind="ExternalInput")
    oh_d = nc.dram_tensor("onehot", [H, 2 * H + 1], f32r, kind="ExternalInput")
    wst_d = nc.dram_tensor("wstopt", [H, 2], f32, kind="ExternalInput")
    b1_d = nc.dram_tensor("bias1", [H, 1], f32, kind="ExternalInput")
    b2_d = nc.dram_tensor("bias2", [H, 1], f32, kind="ExternalInput")
    il_d = nc.dram_tensor("invlen2", [2, BC], f32, kind="ExternalInput")
    id2_d = nc.dram_tensor("ident2", [2, 2], f32, kind="ExternalInput")
    out_d = nc.dram_tensor("out", [BC, L + 1], f32, kind="ExternalOutput")

    SPB = 2 if probe in ("spb2", "spb2s") else SEGS_PER_BLOCK
    blocks = [(s, min(SPB, BC - s)) for s in range(0, BC, SPB)]
    nblk = len(blocks)

    with tile.TileContext(nc) as tc:
        with (
            tc.tile_pool(name="singles", bufs=1) as singles,
            tc.tile_pool(name="xp", bufs=4 if probe == "xp4" else 3) as xp,
            tc.tile_pool(name="h1p", bufs=3 if probe == "h1p3" else 2) as h1p,
            tc.tile_pool(name="h2p", bufs=4 if probe == "h2p4" else 3) as h2p,
            tc.tile_pool(name="gb", bufs=2) as gb,
            tc.tile_pool(name="p1", bufs=2 if probe == "spb2" else 1,
                         space="PSUM") as p1,
            tc.tile_pool(name="p2", bufs=1, space="PSUM") as p2,
            tc.tile_pool(name="psc", bufs=2, space="PSUM") as psc,
        ):
            # --- persistent weights/constants ---
            a2 = singles.tile([2, H], bf16)
            w2t = singles.tile([H, H], f32r)
            oh = singles.tile([H, 2 * H + 1], f32r)
            wst = singles.tile([H, 2], f32)
            b1t = singles.tile([H, 1], f32)
            b2t = singles.tile([H, 1], f32)
            il2 = singles.tile([2, BC], f32)
            id2 = singles.tile([2, 2], f32)
            ones21 = singles.tile([2, 1], f32)
            ones12 = singles.tile([1, 2], f32)
            # small/urgent first; the big w2t/onehot transfers are emitted
            # after the first X blocks so mm1(0) isn't queued behind them
            # (w2t is first needed by mm2 at emission step 1, onehot by the
            # score matmuls at step 2)
            nc.sync.dma_start(a2[:], a_d[:])
            nc.sync.dma_start(b1t[:], b1_d[:])
            nc.sync.dma_start(b2t[:], b2_d[:])
            nc.sync.dma_start(wst[:], wst_d[:])
            nc.sync.dma_start(il2[:], il_d[:])
            nc.sync.dma_start(id2[:], id2_d[:])
            nc.vector.memset(ones21[:], 1.0)
            nc.vector.memset(ones12[:], 1.0)
            # prologue warmers: trigger the Silu ACT-table load and the PE
            # HAM un-throttle while the first X block is still in flight
            # (both would otherwise land on the critical path at t~3us)
            wtab = singles.tile([1, 8], f32)
            nc.vector.memset(wtab[:], 0.0)
            nc.scalar.activation(wtab[:], wtab[:], AF.Silu)

            # per-emission-stage state
            h1t_of = {}
            h2t_of = {}
            gstate = {}
            done_groups = []
            rep = 0

            def get_group(g):
                key = (rep, g)
                if key not in gstate:
                    gstate[key] = {
                        "sacc": psc.tile([GROUP, L], f32, tag="sacc",
                                         name=f"sacc{rep}_{g}"),
                        "ssum": gb.tile([H, GROUP], f32, tag="ssum",
                                        name=f"ssum{rep}_{g}"),
                    }
                return gstate[key]

            def stage1(i):
                """DMA X block, layer-1 matmul, swish1."""
                seg0, n = blocks[i]
                xt = xp.tile([2, SPB * L], bf16, tag="xt",
                             name=f"xt{rep}_{i}")
                nc.gpsimd.dma_start(xt[:, 0:n * L],
                                    x_d[:, seg0 * L:(seg0 + n) * L])
                h1pre = p1.tile([H, SPB * L], f32, tag="h1pre",
                                name=f"h1pre{rep}_{i}")
                for j in range(n):
                    nc.tensor.matmul(h1pre[:, j * L:(j + 1) * L], a2[:],
                                     xt[:, j * L:(j + 1) * L],
                                     start=True, stop=True)
                h1t = h1p.tile([H, SPB * L], f32r, tag="h1t",
                               name=f"h1t{rep}_{i}")
                aw = 128 if probe == "small_act" else n * L
                if probe == "act_split":
                    h_ = aw // 2
                    nc.scalar.activation(h1t[:, 0:h_], h1pre[:, 0:h_],
                                         AF.Silu, bias=b1t[:])
                    nc.scalar.activation(h1t[:, h_:aw], h1pre[:, h_:aw],
                                         AF.Silu, bias=b1t[:])
                else:
                    nc.scalar.activation(h1t[:, 0:aw], h1pre[:, 0:aw],
                                         AF.Silu, bias=b1t[:])
                if probe == "act2":
                    sc1 = h1p.tile([H, SPB * L], f32r, tag="actscratch",
                                   name=f"actsc{rep}_{i}")
                    nc.scalar.activation(sc1[:, 0:n * L], h1pre[:, 0:n * L],
                                         AF.Silu, bias=b1t[:])
                if probe == "pe2":
                    for j in range(n):
                        nc.tensor.matmul(h1pre[:, j * L:(j + 1) * L], a2[:],
                                         xt[:, j * L:(j + 1) * L],
                                         start=True, stop=True)
                if probe == "dma2":
                    sc2 = xp.tile([2, SPB * L], bf16, tag="dmascratch",
                                  name=f"dmasc{rep}_{i}")
                    nc.sync.dma_start(sc2[:, 0:n * L],
                                      x_d[:, seg0 * L:(seg0 + n) * L])
                h1t_of[i] = h1t

            def stage2(i):
                """layer-2 matmul, swish2, mean-pool partial reduce."""
                seg0, n = blocks[i]
                h1t = h1t_of.pop(i)
                h2pre = p2.tile([H, SPB * L], f32, tag="h2pre",
                                name=f"h2pre{rep}_{i}")
                for j in range(n):
                    nc.tensor.matmul(h2pre[:, j * L:(j + 1) * L], w2t[:],
                                     h1t[:, j * L:(j + 1) * L],
                                     start=True, stop=True)
                h2t = h2p.tile([H, SPB, L], f32r, tag="h2t",
                               name=f"h2t{rep}_{i}")
                if probe == "small_act":
                    nc.scalar.activation(h2t[:, 0, 0:128],
                                         h2pre[:, 0:128], AF.Silu,
                                         bias=b2t[:])
                elif probe == "act_split":
                    h_ = n * L // 2
                    fl = h2t[:].rearrange("p a b -> p (a b)")
                    nc.scalar.activation(fl[:, 0:h_], h2pre[:, 0:h_],
                                         AF.Silu, bias=b2t[:])
                    nc.scalar.activation(fl[:, h_:n * L], h2pre[:, h_:n * L],
                                         AF.Silu, bias=b2t[:])
                else:
                    nc.scalar.activation(
                        h2t[:, 0:n, :].rearrange("p a b -> p (a b)"),
                        h2pre[:, 0:n * L], AF.Silu, bias=b2t[:])
                # windowed mean-pool partial sums, split at group boundaries:
                # reduce [H, cnt, mean_w] -> [H, cnt]
                j0 = 0
                while j0 < n:
                    g = (seg0 + j0) // GROUP
                    jend = min(n, (g + 1) * GROUP - seg0)
                    r = (seg0 + j0) % GROUP
                    st = get_group(g)
                    nc.vector.tensor_reduce(
                        st["ssum"][:, r:r + (jend - j0)],
                        h2t[:, j0:jend, 0:mean_w].bitcast(f32),
                        mybir.AxisListType.X, AL.add)
                    j0 = jend
                if probe == "dve2":
                    dsc = gb.tile([H, SPB], f32, tag="dvescratch",
                                  name=f"dvesc{rep}_{i}")
                    nc.vector.tensor_reduce(
                        dsc[:, 0:n],
                        h2t[:, 0:n, 0:mean_w].bitcast(f32),
                        mybir.AxisListType.X, AL.add)
                h2t_of[i] = h2t

            def stage3(i):
                """node-score matmuls, accumulated segment-major."""
                seg0, n = blocks[i]
                h2t = h2t_of.pop(i)
                for j in range(n):
                    seg = seg0 + j
                    g = seg // GROUP
                    r = seg % GROUP
                    st = get_group(g)
                    if probe == "no_scores" and r not in (0, GROUP - 1):
                        pass
                    else:
                        rr = 0 if probe == "fixedsc" else r
                        nc.tensor.matmul(
                            st["sacc"][:], oh[:, H - rr:2 * H - rr],
                            h2t[:, j, :],
                            start=(r == 0), stop=(r == GROUP - 1),
                            skip_group_check=True)
                    if r == GROUP - 1:
                        if probe == "midepi":
                            epilogue(g)
                        else:
                            epiA(g)
                            done_groups.append(g)

            def epilogue(g):
                st = gstate.pop((rep, g))
                sacc, ssum = st["sacc"], st["ssum"]
                # node softmax over each segment row
                rmax = gb.tile([GROUP, 1], f32, tag="rmax", name=f"rmax{rep}_{g}")
                nc.vector.tensor_reduce(rmax[:], sacc[:],
                                        mybir.AxisListType.X, AL.max)
                scn = gb.tile([GROUP, L], f32, tag="scn", name=f"scn{rep}_{g}")
                nc.vector.tensor_scalar_sub(scn[:], sacc[:], rmax[:])
                esc = gb.tile([GROUP, L], f32, tag="esc", name=f"esc{rep}_{g}")
                esum = gb.tile([GROUP, 1], f32, tag="esum", name=f"esum{rep}_{g}")
                nc.scalar.activation(esc[:], scn[:], AF.Exp,
                                     accum_out=esum[:])
                lse = gb.tile([GROUP, 1], f32, tag="lse", name=f"lse{rep}_{g}")
                nc.scalar.activation(lse[:], esum[:], AF.Ln)
                # stopping head: raw scores, per-segment mean, 2-way lsm
                srw = psc.tile([2, GROUP], f32, tag="sacc", name=f"srw{rep}_{g}")
                nc.tensor.matmul(srw[:], wst[:], ssum[:], start=True,
                                 stop=True)
                sv = gb.tile([2, GROUP], f32, tag="sv", name=f"sv{rep}_{g}")
                nc.vector.tensor_mul(sv[:], srw[:],
                                     il2[:, g * GROUP:(g + 1) * GROUP])
                ee = gb.tile([2, GROUP], f32, tag="ee", name=f"ee{rep}_{g}")
                nc.scalar.activation(ee[:], sv[:], AF.Exp)
                es2 = psc.tile([1, GROUP], f32, tag="sacc", name=f"es2{rep}_{g}")
                nc.tensor.matmul(es2[:], ones21[:], ee[:], start=True,
                                 stop=True)
                ls2 = gb.tile([1, GROUP], f32, tag="ls2", name=f"ls2{rep}_{g}")
                nc.scalar.activation(ls2[:], es2[:], AF.Ln)
                lse2 = psc.tile([2, GROUP], f32, tag="sacc", name=f"lse2{rep}_{g}")
                nc.tensor.matmul(lse2[:], ones12[:], ls2[:], start=True,
                                 stop=True)
                lall = gb.tile([2, GROUP], f32, tag="lall", name=f"lall{rep}_{g}")
                nc.vector.tensor_sub(lall[:], sv[:], lse2[:])
                lallt = psc.tile([GROUP, 2], f32, tag="sacc", name=f"lallt{rep}_{g}")
                nc.tensor.transpose(lallt[:], lall[:], id2[:])
                # out[:, :L] = scn - (lse - l0); out[:, L] = l1
                cc = gb.tile([GROUP, 1], f32, tag="cc", name=f"cc{rep}_{g}")
                nc.vector.tensor_sub(cc[:], lse[:], lallt[:, 0:1])
                ot = gb.tile([GROUP, L + 1], f32, tag="ot", name=f"ot{rep}_{g}")
                nc.vector.tensor_scalar_sub(ot[:, 0:L], scn[:], cc[:])
                nc.vector.tensor_copy(ot[:, L:L + 1], lallt[:, 1:2])
                nc.sync.dma_start(out_d[g * GROUP:(g + 1) * GROUP, :], ot[:])

            # software-pipelined emission: layer2 lags 1 block,
            # scores lag 2 blocks (keeps PE fed without stalling ACT)
            epiA_state = {}

            def epiA(g):
                """non-ACT epilogue head: runs in pipeline slack as soon as
                the group's scores finish (no table switch)."""
                st = gstate.pop((rep, g))
                sacc, ssum = st["sacc"], st["ssum"]
                rmax = gb.tile([GROUP, 1], f32, tag="rmax",
                               name=f"ermax{rep}_{g}")
                nc.vector.tensor_reduce(rmax[:], sacc[:],
                                        mybir.AxisListType.X, AL.max)
                scn = gb.tile([GROUP, L], f32, tag="scn",
                              name=f"escn{rep}_{g}")
                nc.vector.tensor_scalar_sub(scn[:], sacc[:], rmax[:])
                srw = psc.tile([2, GROUP], f32, tag="sacc",
                               name=f"esrw{rep}_{g}")
                nc.tensor.matmul(srw[:], wst[:], ssum[:], start=True,
                                 stop=True)
                sv = gb.tile([2, GROUP], f32, tag="sv",
                             name=f"esv{rep}_{g}")
                nc.vector.tensor_mul(sv[:], srw[:],
                                     il2[:, g * GROUP:(g + 1) * GROUP])
                epiA_state[g] = (scn, sv)

            def epilogue_end():
                t_ = {}
                for g in done_groups:
                    t_[g] = epiA_state.pop(g)
                # batched Exp ops, then batched Ln ops: one table
                # neighborhood switch instead of one per group
                for g in done_groups:
                    scn, sv = t_[g]
                    esc = gb.tile([GROUP, L], f32, tag="esc",
                                  name=f"eesc{rep}_{g}")
                    esum = gb.tile([GROUP, 1], f32, tag="esum",
                                   name=f"eesum{rep}_{g}")
                    nc.scalar.activation(esc[:], scn[:], AF.Exp,
                                         accum_out=esum[:])
                    ee = gb.tile([2, GROUP], f32, tag="ee",
                                 name=f"eee{rep}_{g}")
                    nc.scalar.activation(ee[:], sv[:], AF.Exp)
                    es2 = psc.tile([1, GROUP], f32, tag="sacc",
                                   name=f"ees2{rep}_{g}")
                    nc.tensor.matmul(es2[:], ones21[:], ee[:], start=True,
                                     stop=True)
                    t_[g] = (scn, sv, esum, es2)
                for g in done_groups:
                    scn, sv, esum, es2 = t_[g]
                    lse = gb.tile([GROUP, 1], f32, tag="lse",
                                  name=f"else{rep}_{g}")
                    nc.scalar.activation(lse[:], esum[:], AF.Ln)
                    ls2 = gb.tile([1, GROUP], f32, tag="ls2",
                                  name=f"els2{rep}_{g}")
                    nc.scalar.activation(ls2[:], es2[:], AF.Ln)
                    lse2 = psc.tile([2, GROUP], f32, tag="sacc",
                                    name=f"else2{rep}_{g}")
                    nc.tensor.matmul(lse2[:], ones12[:], ls2[:], start=True,
                                     stop=True)
                    lall = gb.tile([2, GROUP], f32, tag="lall",
                                   name=f"elall{rep}_{g}")
                    nc.vector.tensor_sub(lall[:], sv[:], lse2[:])
                    lallt = psc.tile([GROUP, 2], f32, tag="sacc",
                                     name=f"elallt{rep}_{g}")
                    nc.tensor.transpose(lallt[:], lall[:], id2[:])
                    cc = gb.tile([GROUP, 1], f32, tag="cc",
                                 name=f"ecc{rep}_{g}")
                    nc.vector.tensor_sub(cc[:], lse[:], lallt[:, 0:1])
                    ot = gb.tile([GROUP, L + 1], f32, tag="ot",
                                 name=f"eot{rep}_{g}")
                    nc.vector.tensor_scalar_sub(ot[:, 0:L], scn[:], cc[:])
                    nc.vector.tensor_copy(ot[:, L:L + 1], lallt[:, 1:2])
                    nc.sync.dma_start(out_d[g * GROUP:(g + 1) * GROUP, :],
                                      ot[:])
                done_groups.clear()

            for rep in range(nreps):
                for eb in range(nblk + 2):
                    if eb < nblk:
                        stage1(eb)
                    if rep == 0 and eb == 0:
                        nc.sync.dma_start(w2t[:], w2t_d[:])
                    elif rep == 0 and eb == 1:
                        nc.sync.dma_start(oh[:], oh_d[:])
                    if 1 <= eb < nblk + 1:
                        stage2(eb - 1)
                    if 2 <= eb:
                        stage3(eb - 2)
                if probe != "midepi":
                    epilogue_end()

    nc.compile()
    return nc


def _get_program(mean_w, nreps=1, probe=None):
    key = (mean_w, nreps, probe)
    if key not in _nc_cache:
        _nc_cache[key] = _build(mean_w, nreps, probe)
    return _nc_cache[key]


def kernel(x_seeds, x_nodes, w_seed, w_node, W1, b1, W2, b2, w_score, W_stop,
           indptr):
    x_seeds = np.asarray(x_seeds, dtype=np.float32)
    x_nodes = np.asarray(x_nodes, dtype=np.float32)
    w_seed = np.asarray(w_seed, dtype=np.float32)
    w_node = np.asarray(w_node, dtype=np.float32)
    W1 = np.asarray(W1, dtype=np.float32)
    b1 = np.asarray(b1, dtype=np.float32)
    W2 = np.asarray(W2, dtype=np.float32)
    b2 = np.asarray(b2, dtype=np.float32)
    w_score = np.asarray(w_score, dtype=np.float32)
    W_stop = np.asarray(W_stop, dtype=np.float32)
    indptr = np.asarray(indptr)

    starts = indptr[:, 0].astype(np.int64)
    mean_len = (indptr[:, 1] - indptr[:, 0]).astype(np.int64)
    cand_len = (indptr[:, 2] - indptr[:, 0]).astype(np.int64)
    regular = (
        x_seeds.shape == (N,)
        and indptr.shape == (B, 3)
        and np.array_equal(starts, np.arange(B, dtype=np.int64) * L)
        and np.all(cand_len == L)
        and np.all(mean_len == mean_len[0])
        and 1 <= mean_len[0] <= L
    )
    if not regular:
        return _numpy_ref(x_seeds, x_nodes, w_seed, w_node, W1, b1, W2, b2,
                          w_score, W_stop, indptr)

    mean_w = int(mean_len[0])
    from concourse.bass_utils import run_bass_kernel_spmd

    nc = _get_program(mean_w)

    import ml_dtypes
    # host-side folds (cheap, O(H^2))
    A2 = np.stack([W1 @ w_seed, W1 @ w_node]).astype(ml_dtypes.bfloat16)
    W2T = np.ascontiguousarray(W2.T)                               # [H,H]
    onehot = np.zeros((H, 2 * H + 1), np.float32)
    onehot[:, H] = w_score
    WstopT = np.ascontiguousarray(W_stop.T)                        # [H,2]
    b1c = np.ascontiguousarray(b1.reshape(H, 1))
    b2c = np.ascontiguousarray(b2.reshape(H, 1))
    id2 = np.eye(2, dtype=np.float32)
    X = np.stack([x_seeds, x_nodes]).astype(ml_dtypes.bfloat16)
    invlen = (1.0 / mean_len.astype(np.float32))                   # [B]

    in_maps = []
    for c in range(NCORES):
        seg0 = c * BC
        in_maps.append({
            "x2": np.ascontiguousarray(X[:, seg0 * L:(seg0 + BC) * L]),
            "a2": A2,
            "w2t": W2T,
            "onehot": onehot,
            "wstopt": WstopT,
            "bias1": b1c,
            "bias2": b2c,
            "invlen2": np.ascontiguousarray(
                np.broadcast_to(invlen[seg0:seg0 + BC], (2, BC))),
            "ident2": id2,
        })

    res = run_bass_kernel_spmd(nc, in_maps, core_ids=list(range(NCORES)))
    out = np.concatenate([res.results[c]["out"] for c in range(NCORES)],
                         axis=0)
    return out


def _prepare(x_seeds, x_nodes, w_seed, w_node, W1, b1, W2, b2, w_score,
             W_stop, indptr):
    """Build (program, in_maps) for the regular fast path. Test-only hook."""
    import ml_dtypes
    indptr = np.asarray(indptr)
    mean_w = int(indptr[0, 1] - indptr[0, 0])
    nc = _get_program(mean_w)
    A2 = np.stack([W1 @ w_seed, W1 @ w_node]).astype(ml_dtypes.bfloat16)
    W2T = np.ascontiguousarray(W2.T.astype(np.float32))
    onehot = np.zeros((H, 2 * H + 1), np.float32)
    onehot[:, H] = w_score
    WstopT = np.ascontiguousarray(W_stop.T.astype(np.float32))
    b1c = np.ascontiguousarray(b1.reshape(H, 1).astype(np.float32))
    b2c = np.ascontiguousarray(b2.reshape(H, 1).astype(np.float32))
    id2 = np.eye(2, dtype=np.float32)
    X = np.stack([x_seeds, x_nodes]).astype(ml_dtypes.bfloat16)
    mean_len = (indptr[:, 1] - indptr[:, 0]).astype(np.float32)
    invlen = 1.0 / mean_len
    in_maps = []
    for c in range(NCORES):
        seg0 = c * BC
        in_maps.append({
            "x2": np.ascontiguousarray(X[:, seg0 * L:(seg0 + BC) * L]),
            "a2": A2, "w2t": W2T, "onehot": onehot, "wstopt": WstopT,
            "bias1": b1c, "bias2": b2c,
            "invlen2": np.ascontiguousarray(
                np.broadcast_to(invlen[seg0:seg0 + BC], (2, BC))),
            "ident2": id2,
        })
    return nc, in_maps

